# revision 11
# baseline (speedup 1.0000x reference)
"""Trainium2 Bass kernel for the ConditioningEncoder GNN message-passing model.

Math restructuring: with edge_fts[i,j,:] = A[i,j]*We0 + adj[i,j]*We1
+ pred[i,j]*We2 + be, the masked aggregation decomposes into
  msg[j,:] = ( sum_i adjself[i,j]*nf[i,:]          # (N,N)@(N,H) stream
             + cA[j]*We0 + cpred[j]*We2 + diag[j]*We1 - We1   # rank-4 corr
             + cdeg[j]*(We1+be) ) / cdeg[j]
where cA/cpred/diag/cdeg are per-node column reductions computed ONCE and
shared by both MP rounds (cadj = cdeg - 1 + diag).  The cdeg*(We1+be) term
divided by cdeg is constant per column, so it folds into an effective MLP
bias: bmp_eff_r = bmp_r + Wmp_b_r^T (We1 + be).  d_delta folds into the
node-feature weights: Wn3eff = [Wn0, Wn1-Wn3, Wn2+Wn3] (built on-chip with
a tiny constant selection matmul).

adj/A load as f32 over HWDGE in 2-plane (adj|A) pair-chunk DMAs and stream
through the PE as float32r (full rate for >=256 columns) -- no cast pass.
st = adjself*A (DVE) and the one-hot pred mask (gpsimd scalar_tensor_tensor)
are bf16.  Column sums of st/pred use 2-col [ones|0]/[0|ones] stationaries
accumulating into one PSUM tile.  Engine APs must start at partition
0/32/64/96, so the rank-4 corr coefficients live at stride-32 partitions of
one tall tile.

Sharding: data-parallel over k (16 examples / 8 cores = 2 per core), params
replicated, on-device AllReduce of the (1,128) partial mean at the end.
"""

import sys

sys.path.insert(0, "/opt/trn_rl_repo")

import numpy as np

import concourse.bass as bass
import concourse.bacc as bacc
import concourse.mybir as mybir
from concourse import tile
from concourse.bass_utils import run_bass_kernel_spmd

K, N, T, H, Z = 16, 512, 8, 64, 128
NCORES = 8
KLOC = K // NCORES  # 2 examples per core
P = 128             # SBUF partitions
NT = N // P         # 4 row-chunks per (N,N) matrix
F32 = mybir.dt.float32
F32R = mybir.dt.float32r
I32 = mybir.dt.int32
BF = mybir.dt.bfloat16
AF = mybir.ActivationFunctionType
OP = mybir.AluOpType


def _encoder(tc: "tile.TileContext", io: dict, collective: bool = True,
             reps: int = 1, interleave: bool = False):
    nc = tc.nc
    with (
        tc.tile_pool(name="const", bufs=1) as cpool,
        tc.tile_pool(name="chk", bufs=8) as chkpool,
        tc.tile_pool(name="keep", bufs=1) as keep,
        tc.tile_pool(name="stage", bufs=3) as stpool,
        tc.tile_pool(name="small", bufs=2) as smpool,
        tc.tile_pool(name="psA", bufs=1, space="PSUM") as psA,
        tc.tile_pool(name="psB", bufs=1, space="PSUM") as psB,
        tc.tile_pool(name="psmm", bufs=3, space="PSUM") as psmm,
        tc.tile_pool(name="dram", bufs=1, space="DRAM") as dpool,
    ):
        def row(ap):  # (X,) dram AP -> (1,X)
            return ap.rearrange("(p j) -> p j", p=1)

        # ---- constants / params (outside the timed loop) ----------------
        ident = cpool.tile([P, P], BF)
        nc.vector.memset(ident[:], 1.0)
        nc.gpsimd.affine_select(
            ident[:], ident[:], pattern=[[1, P]], compare_op=OP.is_equal,
            fill=0.0, base=0, channel_multiplier=-1,
        )
        identF = cpool.tile([P, P], F32)
        nc.vector.memset(identF[:], 1.0)
        nc.gpsimd.affine_select(
            identF[:], identF[:], pattern=[[1, P]], compare_op=OP.is_equal,
            fill=0.0, base=0, channel_multiplier=-1,
        )
        # iota_i[p, t] = p + 128*t  (int32), per-chunk row indices
        iota_i = cpool.tile([P, NT], I32)
        nc.gpsimd.iota(iota_i[:], pattern=[[P, NT]], base=0, channel_multiplier=1)
        onesrow = cpool.tile([1, N], F32R)
        nc.sync.dma_start(onesrow[:], row(io["onesd"][:]))

        # wbig (64,387): [Wmp0a|Wmp0b|Wmp1a|Wmp1b|Wz|bn|bmp0|bmp1]
        wbigF = cpool.tile([H, 387], F32)
        nc.sync.dma_start(wbigF[:], io["wbig"][:, :])
        # Wz with the readout 1/(N*K) mean folded in; f32 so the f32 mrow
        # column can be the stationary in the readout matmul
        WzM = cpool.tile([H, Z], F32)
        nc.scalar.mul(WzM[:], wbigF[:, 4 * H:4 * H + Z], 1.0 / (N * K))
        bn_sb = wbigF[:, 384:385]
        bmp_sb = [wbigF[:, 385:386], wbigF[:, 386:387]]
        # vrow (1,448): [Wn3|We0|We1|We2|be|bz]
        vrowF = cpool.tile([1, 448], F32)
        nc.sync.dma_start(vrowF[:], io["vrow"][:, :])
        vrow = cpool.tile([1, 320], BF)
        nc.scalar.copy(vrow[:], vrowF[:, 0:320])
        We0, We1, We2 = (vrow[:, H:2 * H], vrow[:, 2 * H:3 * H],
                         vrow[:, 3 * H:4 * H])
        bz_sb = vrowF[:, 320:448]
        # Wn3eff = mpack^T @ Wn = [Wn0, Wn1-Wn3, Wn2+Wn3]  (folds d_delta)
        WnF = cpool.tile([4, H], F32)
        nc.sync.dma_start(WnF[:], io["Wn"][:, :])
        mpF = cpool.tile([4, 3], F32)
        nc.sync.dma_start(mpF[:], io["mpack"][:, :])
        wn3ps = psmm.tile([3, H], F32, tag="mm", bufs=3)
        nc.tensor.matmul(wn3ps[:], mpF[:], WnF[:], start=True, stop=True)
        Wn3e = cpool.tile([3, H], F32R)
        nc.scalar.copy(Wn3e[:], wn3ps[:])
        # wmp (128,128) = [Wmp0 | Wmp1] stacked [a;b] on partitions
        wmpF = cpool.tile([P, 2 * H], F32)
        nc.sync.dma_start(wmpF[:], io["wmp"][:, :])
        wmpS = cpool.tile([P, 2 * H], BF)
        nc.vector.tensor_copy(wmpS[:], wmpF[:])
        WmpB = cpool.tile([H, 2 * H], BF)
        nc.vector.tensor_copy(WmpB[:], wmpF[H:2 * H, :])

        # corr terms accumulate during streaming: st x (We0 broadcast to all
        # 128 contraction rows) = We0 (x) cA, likewise pred x We2bc; diag and
        # the constant -We1 column enter as early rank-1 matmuls
        negWe1 = cpool.tile([1, H], F32R)
        nc.vector.tensor_scalar_mul(negWe1[:], vrowF[:, 2 * H:3 * H], -1.0)
        We1r = cpool.tile([1, H], F32R)
        nc.vector.tensor_copy(We1r[:], vrowF[:, 2 * H:3 * H])
        We0bc = cpool.tile([P, H], BF)
        nc.gpsimd.partition_broadcast(We0bc[:], We0, channels=P)
        We2bc = cpool.tile([P, H], BF)
        nc.gpsimd.partition_broadcast(We2bc[:], We2, channels=P)

        # bmp_eff_r = bmp_r + Wmp_b_r^T (We1 + be)
        vWe1be = cpool.tile([1, H], BF)
        nc.vector.tensor_add(vWe1be[:], We1, vrow[:, 4 * H:5 * H])
        vrow_ps = psmm.tile([H, 1], BF, tag="mm", bufs=3)
        nc.tensor.transpose(vrow_ps[:], vWe1be[:], ident[0:1, 0:1])
        vcolS = cpool.tile([H, 1], BF)
        nc.scalar.copy(vcolS[:], vrow_ps[:])
        bmp_eff = []
        for r in range(2):
            bp = psmm.tile([H, 1], F32, tag="mm", bufs=3)
            nc.tensor.matmul(bp[:], WmpB[:, r * H:(r + 1) * H], vcolS[:],
                             start=True, stop=True)
            be_sb = cpool.tile([H, 1], F32, name=f"bmpe{r}")
            nc.scalar.activation(be_sb[:], bp[:], AF.Identity, bias=bmp_sb[r])
            bmp_eff.append(be_sb)

        # nf0-natural stationaries with a persistent ones column per chunk
        nfN0 = []
        for ex in range(KLOC):
            t = keep.tile([P, NT * 65], F32R, tag="nfN0", bufs=2,
                          name=f"nfN0_{ex}")
            tf = t[:]
            dst = bass.AP(tf.tensor, tf.offset + H, [list(tf.ap[0]), [65, NT]])
            nc.sync.dma_start(dst, bass.AP(io["onesd"], 0, [[0, P], [0, NT]]))
            nfN0.append(t)

        import contextlib
        loop_ctx = (tc.For_i(0, reps, 1) if reps > 1
                    else contextlib.nullcontext())
        with loop_ctx:
            S = [dict() for _ in range(KLOC)]
            accA = [psA.tile([65, N], F32, name=f"accA{ex}", tag=f"accA{ex}",
                             bufs=1) for ex in range(KLOC)]
            accD = [psA.tile([H, N], F32, name=f"accD{ex}", tag=f"accD{ex}",
                             bufs=1) for ex in range(KLOC)]

            def chunk_dma(ex, c):
                t = chkpool.tile([P, 2 * N], F32R, tag="chk", bufs=8,
                                 name=f"chk{ex}_{c}")
                nc.sync.dma_start(
                    t[:], bass.AP(io["adjA"], ex * 2 * N * N + c * P * N,
                                  [[N, P], [N * N, 2], [1, N]]))
                S[ex][f"chk{c}"] = t

            def small_dmas(ex):
                s = S[ex]
                sdd = smpool.tile([3, N], F32R, tag="sdd", name=f"sdd{ex}")
                nc.sync.dma_start(sdd[:], io["sdd"][ex])
                pi_i = smpool.tile([1, N], I32, tag="pii", name=f"pii{ex}")
                nc.sync.dma_start(pi_i[:], row(io["piT"][ex]))
                diagF = smpool.tile([1, N], F32R, tag="diagF",
                                    name=f"diagF{ex}")
                nc.sync.dma_start(
                    diagF[:],
                    bass.AP(io["adjA"], ex * 2 * N * N, [[0, 1], [N + 1, N]]))
                s.update(sdd=sdd, pi_i=pi_i, diagF=diagF)

            chunk_dma(0, 0)
            chunk_dma(1, 0)
            small_dmas(0)
            small_dmas(1)
            for c in range(1, NT):
                chunk_dma(0, c)
                chunk_dma(1, c)

            # ---- phase A: node features (both examples) ----
            for ex in range(KLOC):
                s = S[ex]
                pib = keep.tile([P, N], I32, tag="pib", bufs=2,
                                name=f"pib{ex}")
                nc.gpsimd.partition_broadcast(pib[:], s["pi_i"][:], channels=P)
                s["pib"] = pib
                nf0ps = psmm.tile([H, N], F32, tag="mm", bufs=3)
                nc.tensor.matmul(nf0ps[:], Wn3e[:], s["sdd"][:],
                                 start=True, stop=True)
                mlp0 = keep.tile([P, N], BF, tag="mlp0", bufs=2,
                                 name=f"mlp0_{ex}")
                nc.scalar.activation(mlp0[0:H, :], nf0ps[:], AF.Identity,
                                     bias=bn_sb)
                s["mlp0"] = mlp0
                for c in range(NT):
                    tp = psmm.tile([P, H], BF, tag="mm", bufs=3)
                    nc.tensor.transpose(tp[:], mlp0[0:H, c * P:(c + 1) * P],
                                        ident[0:H, 0:H])
                    nc.scalar.copy(nfN0[ex][:, c * 65:c * 65 + H], tp[:])

            # early corr terms: diag (x) We1 and the constant -We1 column
            for ex in range(KLOC):
                nc.tensor.matmul(accD[ex][:], We1r[:], S[ex]["diagF"][:],
                                 start=True, stop=False)
                nc.tensor.matmul(accD[ex][:], negWe1[:], onesrow[:],
                                 start=False, stop=False)

            # ---- phase B: per-chunk pipeline ----
            for c in range(NT):
                for ex in range(KLOC):
                    s = S[ex]
                    chk = s[f"chk{c}"]
                    # adj_self: add I on the diagonal block (in place, f32)
                    nc.vector.tensor_add(chk[:, c * P:(c + 1) * P],
                                         chk[:, c * P:(c + 1) * P], identF[:])
                    st = stpool.tile([P, N], BF, tag="st", bufs=3,
                                     name=f"st{ex}_{c}")
                    nc.vector.tensor_tensor(st[:], chk[:, 0:N], chk[:, N:2 * N],
                                            op=OP.mult)
                    pred = stpool.tile([P, N], BF, tag="pred", bufs=3,
                                       name=f"pred{ex}_{c}")
                    nc.vector.scalar_tensor_tensor(
                        pred[:], s["pib"][:], iota_i[:, c:c + 1], chk[:, 0:N],
                        op0=OP.is_equal, op1=OP.mult)
                    adjr = chk[:, 0:N]
                    s[f"adjr{c}"] = adjr
                    nc.tensor.matmul(accA[ex][0:65, :],
                                     nfN0[ex][:, c * 65:(c + 1) * 65],
                                     adjr, start=(c == 0), stop=(c == NT - 1))
                    nc.tensor.matmul(accD[ex][:], We0bc[:], st[:],
                                     start=False, stop=False)
                    nc.tensor.matmul(accD[ex][:], We2bc[:], pred[:],
                                     start=False, stop=(c == NT - 1))

            # ---- phase C: per-example finalize, step-interleaved ----
            ez2 = psmm.tile([1, KLOC * Z], F32, tag="ez", bufs=1)
            invb = []
            for ex in range(KLOC):
                # WAR: reciprocal reads accA row 64 before corr writes rows 0:64
                invd = smpool.tile([1, N], F32, tag="invd", name=f"invd{ex}")
                nc.vector.reciprocal(invd[:], accA[ex][64:65, :])
                ib = keep.tile([H, N], F32, tag="invb", bufs=2,
                               name=f"invb{ex}")
                nc.gpsimd.partition_broadcast(ib[:], invd[:], channels=H)
                invb.append(ib)
                cS = stpool.tile([H, N], BF, tag="corrS", bufs=2,
                                 name=f"corrS{ex}")
                nc.scalar.copy(cS[:], accD[ex][:])
                S[ex]["corrS"] = cS
            for ex in range(KLOC):
                nc.tensor.matmul(accA[ex][0:64, :], ident[0:H, 0:H],
                                 S[ex]["corrS"][:],
                                 start=False, stop=True, skip_group_check=True)
            mlp1 = []
            for ex in range(KLOC):
                nc.vector.tensor_tensor(S[ex]["mlp0"][H:P, :],
                                        accA[ex][0:H, :], invb[ex][:],
                                        op=OP.mult)
            for ex in range(KLOC):
                psX = psmm.tile([H, N], F32, tag="mm", bufs=3)
                nc.tensor.matmul(psX[:], wmpS[:, 0:H], S[ex]["mlp0"][:],
                                 start=True, stop=True)
                m1 = keep.tile([P, N], BF, tag="mlp1", bufs=2,
                               name=f"mlp1_{ex}")
                nc.scalar.activation(m1[0:H, :], psX[:], AF.Relu,
                                     bias=bmp_eff[0][:])
                mlp1.append(m1)
            nfN1 = []
            for ex in range(KLOC):
                n1 = keep.tile([P, NT * H], F32R, tag="nfN1", bufs=2,
                               name=f"nfN1_{ex}")
                for c in range(NT):
                    tp = psmm.tile([P, H], BF, tag="mm", bufs=3)
                    nc.tensor.transpose(tp[:], mlp1[ex][0:H, c * P:(c + 1) * P],
                                        ident[0:H, 0:H])
                    nc.scalar.copy(n1[:, c * H:(c + 1) * H], tp[:])
                nfN1.append(n1)
            accC = []
            for ex in range(KLOC):
                aC = psmm.tile([H, N], F32, tag="mm", bufs=3)
                for c in range(NT):
                    nc.tensor.matmul(aC[:], nfN1[ex][:, c * H:(c + 1) * H],
                                     S[ex][f"adjr{c}"],
                                     start=(c == 0), stop=(c == NT - 1))
                nc.tensor.matmul(aC[:], ident[0:H, 0:H], S[ex]["corrS"][:],
                                 start=False, stop=True, skip_group_check=True)
                accC.append(aC)
            for ex in range(KLOC):
                nc.vector.tensor_tensor(mlp1[ex][H:P, :], accC[ex][:],
                                        invb[ex][:], op=OP.mult)
            for ex in range(KLOC):
                psY = psmm.tile([H, N], F32, tag="mm", bufs=3)
                nc.tensor.matmul(psY[:], wmpS[:, H:2 * H], mlp1[ex][:],
                                 start=True, stop=True)
                nfT2 = stpool.tile([H, N], BF, tag="nfT2", bufs=2,
                                   name=f"nfT2_{ex}")
                mrow = smpool.tile([H, 1], F32, tag="mrow", name=f"mrow{ex}")
                nc.scalar.activation(nfT2[:], psY[:], AF.Relu,
                                     bias=bmp_eff[1][:], accum_out=mrow[:])
                nc.tensor.matmul(ez2[:, ex * Z:(ex + 1) * Z], mrow[:], WzM[:],
                                 start=True, stop=True)

            zacc = smpool.tile([1, Z], F32, tag="zacc",
                               bufs=(1 if reps == 1 else 2))
            # fold bz/NCORES into every core's partial so the AllReduce sums
            # to exactly one bz
            nc.vector.scalar_tensor_tensor(zacc[:], bz_sb, 1.0 / NCORES,
                                           ez2[:, 0:Z], op0=OP.mult,
                                           op1=OP.add)
            nc.vector.tensor_tensor(zacc[:], zacc[:], ez2[:, Z:2 * Z],
                                    op=OP.add)

        # ---- all-reduce the partial means across cores ------------------
        cc_in = dpool.tile([1, Z], F32, tag="ccin")
        cc_out = dpool.tile([1, Z], F32, tag="ccout",
                            addr_space="Shared" if collective else "Local")
        nc.sync.dma_start(cc_in[:], zacc[:])
        if collective:
            nc.gpsimd.collective_compute(
                "AllReduce", OP.add, replica_groups=[list(range(NCORES))],
                ins=[cc_in.opt()], outs=[cc_out.opt()],
            )
        else:
            nc.gpsimd.dma_start(cc_out[:], cc_in[:])
        nc.sync.dma_start(io["z"][:].rearrange("(p j) -> p j", p=1), cc_out[:])


def build_program(collective: bool = True, reps: int = 1,
                  interleave: bool = False) -> bass.Bass:
    nc = bacc.Bacc("TRN2", target_bir_lowering=False, num_devices=NCORES)
    io = {}
    for name, shape, dt in [
        ("adjA", [KLOC, 2, N, N], F32R),
        ("sdd", [KLOC, 3, N], F32R),
        ("onesd", [N], F32R),
        ("piT", [KLOC, N], I32), ("Wn", [4, H], F32),
        ("wbig", [H, 387], F32), ("vrow", [1, 448], F32),
        ("wmp", [P, 2 * H], F32), ("mpack", [4, 3], F32),
    ]:
        io[name] = nc.dram_tensor(name, shape, dt, kind="ExternalInput")
    io["z"] = nc.dram_tensor("z", [Z], F32, kind="ExternalOutput")
    with tile.TileContext(nc) as tc:
        _encoder(tc, io, collective=collective, reps=reps,
                 interleave=interleave)
    nc.compile()
    return nc


_PROGRAM = None


def _get_program():
    global _PROGRAM
    if _PROGRAM is None:
        _PROGRAM = build_program()
    return _PROGRAM


MPACK = np.array([[1, 0, 0], [0, 1, 0], [0, 0, 1], [0, -1, 1]], np.float32)


def make_in_maps(s, A, adj, d_hints, pi_hints, Wn, bn, We, be,
                 Wmp0, bmp0, Wmp1, bmp1, Wz, bz):
    f32 = lambda x: np.ascontiguousarray(x, np.float32)
    # host-side packing is layout-only (concatenation of replicated params
    # and per-example row slices); mpack is a compile-time constant
    wbig = np.concatenate(
        [f32(Wmp0[:H]), f32(Wmp0[H:]), f32(Wmp1[:H]), f32(Wmp1[H:]),
         f32(Wz), f32(bn)[:, None], f32(bmp0)[:, None], f32(bmp1)[:, None]],
        axis=1)
    vrow = np.concatenate(
        [f32(Wn[3]), f32(We[0]), f32(We[1]), f32(We[2]), f32(be),
         f32(bz)])[None, :]
    wmp = np.concatenate([f32(Wmp0), f32(Wmp1)], axis=1)
    d0, dT = d_hints[0], d_hints[-1]
    sdd = np.stack([f32(s), f32(d0), f32(dT)], axis=1)          # (K,3,N)
    adjA = np.stack([f32(adj), f32(A)], axis=1)                 # (K,2,N,N)
    params = dict(Wn=f32(Wn), wbig=f32(wbig), vrow=f32(vrow), wmp=f32(wmp),
                  mpack=MPACK, onesd=np.ones(N, np.float32))
    in_maps = []
    for c in range(NCORES):
        ks = slice(c * KLOC, (c + 1) * KLOC)
        in_maps.append(dict(
            adjA=np.ascontiguousarray(adjA[ks]),
            sdd=np.ascontiguousarray(sdd[ks]),
            piT=np.ascontiguousarray(pi_hints[-1, ks], np.int32),
            **params,
        ))
    return in_maps


def kernel(s, A, adj, d_hints, pi_hints, Wn, bn, We, be,
           Wmp0, bmp0, Wmp1, bmp1, Wz, bz, **run_kwargs):
    args = [np.asarray(x) for x in (s, A, adj, d_hints, pi_hints, Wn, bn,
                                    We, be, Wmp0, bmp0, Wmp1, bmp1, Wz, bz)]
    nc = _get_program()
    in_maps = make_in_maps(*args)
    res = run_bass_kernel_spmd(nc, in_maps, list(range(NCORES)), **run_kwargs)
    out = np.asarray(res.results[0]["z"], np.float32).reshape(Z)
    if run_kwargs:
        return out, res
    return out


if __name__ == "__main__":
    build_program()
    print("program built OK")
